# revision 12
# baseline (speedup 1.0000x reference)
"""GATv2 (3-layer, 4-head, GraphNorm) Bass kernel for 8 trn2 NeuronCores.

Sharding: nodes partitioned by dst across 8 cores. Each core writes a
combined projection table (xl for all shards + xr for its own shard) to
DRAM with batched DMAs, gathers xl[src] and xr[dst] per 128-dst-node block
via SWDGE dma_gather, sums them with identity matmuls in PSUM, does
block-batched edge math with host-precomputed one-hot dst masks feeding
the segment-softmax aggregation matmuls, then GraphNorm with an AllReduce
for global stats and an AllGather of transposed node-feature shards
feeding the next layer's projections.
"""
import math

import ml_dtypes
import numpy as np

import concourse.bacc as bacc
import concourse.bass as bass
import concourse.tile as tile
from concourse import mybir
from concourse.bass_utils import run_bass_kernel_spmd
from concourse.masks import make_identity

F32 = mybir.dt.float32
BF16 = mybir.dt.bfloat16
I16 = mybir.dt.int16
I32 = mybir.dt.int32
AF = mybir.ActivationFunctionType
ALU = mybir.AluOpType

NC = 8
D = 64
H = 4
C = 64
HC = H * C  # 256
L = 3
NEG = 0.2
EPS = 1e-5
P = 128
NQ = 4  # SWDGE queues (1-4)


def _bf(x):
    return np.asarray(x, dtype=ml_dtypes.bfloat16)


def _wrap_idx(idx):
    """[n*128] int -> [128, n*8] int16 wrapped in 16 partitions, replicated
    across the 8 gpsimd core groups."""
    n = idx.shape[0]
    assert n % 128 == 0
    w = idx.reshape(n // 16, 16).T  # [16, n//16]
    return np.tile(w, (8, 1)).astype(np.int16)


def preprocess(inputs):
    """Host-side: shard/sort/pad edges, build all per-core input tensors."""
    x = np.asarray(inputs["x"], np.float32)
    ei = np.asarray(inputs["edge_index"], np.int64)
    Wl = np.asarray(inputs["Wl"], np.float32)
    bl = np.asarray(inputs["bl"], np.float32)
    Wr = np.asarray(inputs["Wr"], np.float32)
    br = np.asarray(inputs["br"], np.float32)
    att = np.asarray(inputs["att"], np.float32)
    conv_bias = np.asarray(inputs["conv_bias"], np.float32)
    gn_weight = np.asarray(inputs["gn_weight"], np.float32)
    gn_scale = np.asarray(inputs["gn_scale"], np.float32)
    gn_bias = np.asarray(inputs["gn_bias"], np.float32)

    N = x.shape[0]
    NSH = N // NC
    NBLK = (NSH + P - 1) // P
    RW = NBLK * P
    NT = N + (-N) % P

    loop = np.arange(N, dtype=np.int64)
    src = np.concatenate([ei[0], loop])
    dst = np.concatenate([ei[1], loop])

    per_core = []
    cnts = np.zeros((NC, NBLK), np.int64)
    for c in range(NC):
        sel = (dst >= c * NSH) & (dst < (c + 1) * NSH)
        s = src[sel].astype(np.int32)
        dl = (dst[sel] - c * NSH).astype(np.int32)
        order = np.argsort(dl, kind="stable")
        s, dl = s[order], dl[order]
        blk = dl // P
        starts = np.searchsorted(blk, np.arange(NBLK))
        ends = np.searchsorted(blk, np.arange(NBLK), side="right")
        cnts[c] = ends - starts
        per_core.append((s, dl, starts, ends))

    nchunk = [max(1, int(math.ceil(cnts[:, b].max() / P))) for b in range(NBLK)]
    IWC = int(sum(nchunk))
    cum = np.concatenate([[0], np.cumsum(nchunk)]).astype(int)

    iota128 = np.arange(P, dtype=np.int32)
    ONE_BF = np.uint16(0x3F80)  # 1.0 in bf16 bits

    in_maps = []
    for c in range(NC):
        s, dl, starts, ends = per_core[c]
        srcw = np.zeros((P, IWC * 8), np.int16)
        # per block: [s mask (nch*128 cols) | st mask (nch*128 cols)]
        # s[i, k, e] = (dloc[k,e] == i) with partitions=i (dst-local row);
        # st[e, k, i] = same predicate with partitions=e (edge lane)
        stm = np.zeros((P, IWC * 2 * P), np.uint16)
        for b in range(NBLK):
            ns = nchunk[b] * P
            e0, e1 = starts[b], ends[b]
            n = e1 - e0
            sp = np.zeros(ns, np.int16)
            sp[:n] = s[e0:e1]
            lp = np.full(ns, -1, np.int32)  # pad lane: matches no i
            lp[:n] = dl[e0:e1] - b * P
            co = int(cum[b]) * 8
            srcw[:, co : co + nchunk[b] * 8] = _wrap_idx(sp)
            lp_r = lp.reshape(nchunk[b], P)
            m_s = (lp_r[None, :, :] == iota128[:, None, None])  # [i, k, e]
            mo = int(cum[b]) * 2 * P
            stm[:, mo : mo + ns] = m_s.reshape(P, ns) * ONE_BF
            stm[:, mo + ns : mo + 2 * ns] = (
                m_s.transpose(2, 1, 0).reshape(P, ns) * ONE_BF
            )
        in_maps.append(
            {"srcw": srcw, "stm": stm.view(ml_dtypes.bfloat16)}
        )

    # xl carries no bias: bl is folded into the xr-side bias for the logits
    # path (xl+xr unchanged) and into the GraphNorm affine for the
    # aggregation path (mean over heads of bl is a constant shift of h).
    wts = np.zeros((L, 2, D + 1, HC), np.float32)
    for l in range(L):
        wts[l, 0, :D] = Wl[l].T
        wts[l, 1, :D] = Wr[l].T
        wts[l, 1, D] = bl[l] + br[l]
    wts = _bf(wts)

    attb = _bf(att.reshape(L, HC))

    cb_eff = conv_bias + bl.reshape(L, H, C).mean(axis=1)
    gnc = np.stack(
        [
            cb_eff,
            2 * cb_eff,
            cb_eff * cb_eff,
            gn_scale * (2 - gn_scale),
            gn_scale,
            gn_weight,
            gn_bias,
        ],
        axis=1,
    ).astype(np.float32)  # [L, 7, C]

    xt0 = np.zeros((P, 4 * RW), np.float32)
    for r in range(NC):
        hr, ir = r // 4, r % 4
        xt0[hr * D : hr * D + D, ir * RW : ir * RW + NSH] = x[
            r * NSH : (r + 1) * NSH
        ].T
    xt0 = _bf(xt0)

    for c in range(NC):
        in_maps[c]["wts"] = wts
        in_maps[c]["attb"] = attb
        in_maps[c]["gnc"] = gnc
        in_maps[c]["xt0"] = xt0
        xtme = np.zeros((D, RW), np.float32)
        xtme[:, :NSH] = x[c * NSH : (c + 1) * NSH].T
        in_maps[c]["xtme0"] = _bf(xtme)

    cfg = dict(
        N=N, NSH=NSH, NBLK=NBLK, RW=RW, nchunk=[int(v) for v in nchunk],
        cum=[int(v) for v in cum], IWC=IWC,
    )
    return cfg, in_maps


def _ap3(ap, d1, d2):
    """Build [P, d1, d2] AP from a 2D AP by appending explicit dims."""
    return bass.AP(tensor=ap.tensor, offset=ap.offset, ap=[list(ap.ap[0]), d1, d2])


def build(cfg):
    N, NSH, NBLK, RW = cfg["N"], cfg["NSH"], cfg["NBLK"], cfg["RW"]
    nchunk, cum, IWC = cfg["nchunk"], cfg["cum"], cfg["IWC"]
    NT = N + (-N) % P
    nRT = (NSH + P - 1) // P
    NFULL = NSH // P  # full 128-row tiles per shard
    NREM = NSH - NFULL * P  # rows in the last partial tile

    nc = bacc.Bacc(
        "TRN2",
        target_bir_lowering=False,
        debug=False,
        num_devices=NC,
        num_swdge_queues=NQ,
        dynamic_dma_scratch_size=32768,
    )

    srcw = nc.dram_tensor("srcw", [P, IWC * 8], I16, kind="ExternalInput").ap()
    stm = nc.dram_tensor("stm", [P, IWC * 2 * P], BF16, kind="ExternalInput").ap()
    wts = nc.dram_tensor("wts", [L, 2, D + 1, HC], BF16, kind="ExternalInput").ap()
    attb = nc.dram_tensor("attb", [L, HC], BF16, kind="ExternalInput").ap()
    gnc = nc.dram_tensor("gnc", [L, 7, C], F32, kind="ExternalInput").ap()
    xt0 = nc.dram_tensor("xt0", [P, 4 * RW], BF16, kind="ExternalInput").ap()
    xtme0 = nc.dram_tensor("xtme0", [D, RW], BF16, kind="ExternalInput").ap()
    out = nc.dram_tensor("out", [NSH, C], F32, kind="ExternalOutput").ap()

    tab = nc.dram_tensor("tab", [NT, HC], BF16).ap()
    arin = [nc.dram_tensor(f"arin{l}", [P], F32).ap() for l in range(L)]
    arout = [
        nc.dram_tensor(f"arout{l}", [P], F32, addr_space="Shared").ap()
        for l in range(L)
    ]
    agin = [nc.dram_tensor(f"agin{l}", [D, NSH], BF16).ap() for l in range(L - 1)]
    agout = [
        nc.dram_tensor(f"agout{l}", [NC, D, NSH], BF16, addr_space="Shared").ap()
        for l in range(L - 1)
    ]

    def _tab_out_ap(row0, pcnt, nb):
        """DRAM AP over tab rows row0 + b*128 + p (p outer, then b, then c)."""
        return bass.AP(
            tensor=tab.tensor,
            offset=tab.offset + row0 * HC,
            ap=[[HC, pcnt], [P * HC, nb], [1, HC]],
        )

    with tile.TileContext(nc) as tc:
        with (
            tc.tile_pool(name="res", bufs=1) as res,
            tc.tile_pool(name="stg", bufs=2) as stg,
            tc.tile_pool(name="big", bufs=4) as big,
            tc.tile_pool(name="gat", bufs=2) as gat,
            tc.tile_pool(name="idx", bufs=3) as idxp,
            tc.tile_pool(name="msk", bufs=2) as msk,
            tc.tile_pool(name="med", bufs=1) as med,
            tc.tile_pool(name="sm", bufs=2) as sm,
            tc.tile_pool(name="ps", bufs=2, space="PSUM") as ps,
            tc.tile_pool(name="psa", bufs=2, space="PSUM") as psa,
            tc.tile_pool(name="psb", bufs=1, space="PSUM") as psb,
            tc.tile_pool(name="psx", bufs=3, space="PSUM") as psx,
        ):
            # ---- resident loads / constants ----

            ones_row = res.tile([1, P], BF16)
            nc.vector.memset(ones_row[:], 1.0)
            ones_col = res.tile([P, 1], F32)
            nc.vector.memset(ones_col[:], 1.0)
            ident = res.tile([P, P], F32)
            make_identity(nc, ident[:])
            ident_bf = res.tile([P, P], BF16)
            nc.vector.tensor_copy(out=ident_bf[:], in_=ident[:])
            eps_col = res.tile([P, 1], F32)
            nc.vector.memset(eps_col[:], EPS)

            xt_pack = res.tile([P, 4 * RW], BF16)
            nc.sync.dma_start(out=xt_pack[:], in_=xt0[:, :])
            xtme_sb = res.tile([D, RW], BF16)
            nc.sync.dma_start(out=xtme_sb[:], in_=xtme0[:, :])

            w_tiles = {}
            b_tiles = {}
            for l in range(L):
                for side in range(2):
                    # weights duplicated into both partition halves so lhsT
                    # slices based at partition 0 or 64 both find a matching
                    # rhs base
                    t = res.tile([P, HC], BF16, tag=f"w{l}{side}")
                    nc.sync.dma_start(out=t[:D, :], in_=wts[l, side, :D, :])
                    nc.sync.dma_start(out=t[D:, :], in_=wts[l, side, :D, :])
                    w_tiles[(l, side)] = t
                    if side == 1:
                        bt = res.tile([1, HC], BF16, tag=f"b{l}{side}")
                        nc.sync.dma_start(
                            out=bt[:], in_=wts[l, side, D : D + 1, :]
                        )
                        b_tiles[(l, side)] = bt

            att_bc = {}
            for l in range(L):
                t = res.tile([P, HC], BF16, tag=f"att{l}")
                nc.sync.dma_start(
                    out=t[:],
                    in_=bass.AP(
                        tensor=attb.tensor, offset=attb.offset + l * HC,
                        ap=[[0, P], [1, HC]],
                    ),
                )
                att_bc[l] = t

            gnc_bc = {}
            for l in range(L):
                t = res.tile([P, 7, C], F32, tag=f"gnc{l}")
                nc.sync.dma_start(
                    out=t[:],
                    in_=bass.AP(
                        tensor=gnc.tensor, offset=gnc.offset + l * 7 * C,
                        ap=[[0, P], [C, 7], [1, C]],
                    ),
                )
                gnc_bc[l] = t

            xr_res = res.tile([P, NBLK, HC], BF16)
            h_big = res.tile([P, NBLK, C], F32)
            xtsh_sb = res.tile([D, RW], BF16)
            nc.vector.memset(xtsh_sb[:], 0.0)

            for l in range(L):
                # ================= projections =================
                # xr first: it only needs own-shard features (xtme_sb), so on
                # layer boundaries PE runs it while the AllGather that feeds
                # xt_pack (needed by the xl loop below) is still in flight.
                for j in range(nRT):
                    n0 = j * P
                    lhsT = xtme_sb[:, n0 : n0 + P]
                    pt = ps.tile([P, HC], F32, tag="pj", space="PSUM")
                    nc.tensor.matmul(
                        out=pt[:], lhsT=lhsT, rhs=w_tiles[(l, 1)][:D, :],
                        start=True, stop=False,
                    )
                    nc.tensor.matmul(
                        out=pt[:], lhsT=ones_row[:],
                        rhs=b_tiles[(l, 1)][:],
                        start=False, stop=True,
                    )
                    if j % 2 == 0:
                        nc.scalar.activation(xr_res[:, j, :], pt[:], AF.Copy)
                    else:
                        nc.vector.tensor_copy(out=xr_res[:, j, :], in_=pt[:])

                for r in range(NC):
                    hr, ir = r // 4, r % 4
                    xl_st = stg.tile([P, nRT, HC], BF16, tag="pst")
                    for j in range(nRT):
                        n0 = j * P
                        lhsT = xt_pack[
                            hr * D : hr * D + D, ir * RW + n0 : ir * RW + n0 + P
                        ]
                        pt = ps.tile([P, HC], F32, tag="pj", space="PSUM")
                        nc.tensor.matmul(
                            out=pt[:], lhsT=lhsT,
                            rhs=w_tiles[(l, 0)][hr * D : hr * D + D, :],
                            start=True, stop=True,
                        )
                        if j % 2 == 0:
                            nc.scalar.activation(xl_st[:, j, :], pt[:], AF.Copy)
                        else:
                            nc.vector.tensor_copy(out=xl_st[:, j, :], in_=pt[:])
                    g0 = r * NSH
                    nc.sync.dma_start(
                        out=_tab_out_ap(g0, P, NFULL // 2),
                        in_=xl_st[:, : NFULL // 2, :],
                    )
                    nc.sync.dma_start(
                        out=_tab_out_ap(g0 + (NFULL // 2) * P, P, NFULL - NFULL // 2),
                        in_=xl_st[:, NFULL // 2 : NFULL, :],
                    )
                    if NREM:
                        nc.sync.dma_start(
                            out=_tab_out_ap(g0 + NFULL * P, NREM, 1),
                            in_=xl_st[:NREM, NFULL, :],
                        )

                # ================= edge blocks =================
                stats_ps = psb.tile([P, 1], F32, tag="stats", space="PSUM")
                gq = 0
                for b in range(NBLK):
                    nch = nchunk[b]
                    co = cum[b]

                    # gather xl[src] (chunks 0..nch) and xr[dst] (nch..2nch)
                    # dma_gather tops out at 1024 indices -- split into
                    # sub-calls (xl/xr interleaved per 8 chunks, see
                    # preprocess), round-robin across the 4 SWDGE queues
                    xl_g = gat.tile([P, nch, HC], BF16, tag="ug")
                    idx_sb = idxp.tile([P, nch * 8], I16, tag="ix")
                    nc.sync.dma_start(
                        out=idx_sb[:],
                        in_=srcw[:, co * 8 : (co + nch) * 8],
                    )
                    for g in range(0, nch, 8):
                        gn = min(8, nch - g)
                        sub = gn * P
                        nc.gpsimd.dma_gather(
                            out_ap=xl_g[:, g : g + gn, :], in_ap=tab[:, :],
                            idxs_ap=idx_sb[:, g * 8 : (g + gn) * 8],
                            num_idxs=sub, num_idxs_reg=sub, elem_size=HC,
                            queue_num=gq % NQ,
                        )
                        gq += 1

                    # host-precomputed one-hot dst masks: s[i,k,e] | st[e,k,i]
                    mk = msk.tile([P, 2 * nch, P], BF16, tag="mk")
                    mo = co * 2 * P
                    nc.sync.dma_start(
                        out=mk[:, :nch, :],
                        in_=stm[:, mo : mo + nch * P],
                    )
                    nc.sync.dma_start(
                        out=mk[:, nch:, :],
                        in_=stm[:, mo + nch * P : mo + 2 * nch * P],
                    )

                    # u = xl[src] + xr[dst] accumulated in PSUM via identity
                    # matmuls, then leaky-relu straight from PSUM to SBUF
                    lr = big.tile([P, nch, HC], BF16, tag="g")
                    for j in range(0, nch, 2):
                        jn = min(2, nch - j)
                        ups = psx.tile([P, 2, HC], F32, tag="xre", space="PSUM")
                        for t in range(jn):
                            nc.tensor.matmul(
                                out=ups[:, t, :], lhsT=mk[:, j + t, :],
                                rhs=xr_res[:, b, :],
                                start=True, stop=False,
                            )
                            nc.tensor.matmul(
                                out=ups[:, t, :], lhsT=ident_bf[:],
                                rhs=xl_g[:, j + t, :],
                                start=False, stop=True,
                            )
                        nc.scalar.activation(
                            lr[:, j : j + jn, :], ups[:, :jn, :],
                            AF.Prelu, alpha=NEG,
                        )

                    v = big.tile([P, nch, HC], BF16, tag="g")
                    ab = att_bc[l][:]
                    nc.vector.tensor_mul(
                        out=v[:], in0=lr[:], in1=_ap3(ab, [0, nch], [1, HC])
                    )
                    logits = sm.tile([P, nch, H], F32, tag="lg")
                    # [P, nch, H*C] viewed as [P, nch*H, C]: one reduce over C
                    nc.vector.tensor_reduce(
                        out=logits[:].rearrange("p n h -> p (n h)"),
                        in_=v[:].rearrange("p n (h c) -> p (n h) c", h=H),
                        axis=mybir.AxisListType.X,
                        op=ALU.add,
                    )
                    # wcat: cols 0:H hold a=exp(logits), cols H: hold a*xl
                    wcat = big.tile([P, nch, H + HC], BF16, tag="g")
                    nc.scalar.activation(wcat[:, :, :H], logits[:], AF.Exp)
                    # a2: each a value duplicated twice (contiguous) so the
                    # per-head multiply below reads it with an inner [1,2]
                    # stride pattern -> DVE 2x mode
                    a2 = sm.tile([P, nch, H, 2], BF16, tag="a2")
                    lg_ap = logits[:]
                    nc.scalar.activation(
                        a2[:],
                        bass.AP(
                            tensor=lg_ap.tensor, offset=lg_ap.offset,
                            ap=[list(lg_ap.ap[0]), [H, nch], [1, H], [0, 2]],
                        ),
                        AF.Exp,
                    )
                    wc_ap = wcat[:]
                    ug_ap = xl_g[:]
                    a2_ap = a2[:]
                    for h in range(H):
                        nc.vector.tensor_mul(
                            out=bass.AP(
                                tensor=wc_ap.tensor,
                                offset=wc_ap.offset + H + h * C,
                                ap=[list(wc_ap.ap[0]), [H + HC, nch],
                                    [2, C // 2], [1, 2]],
                            ),
                            in0=bass.AP(
                                tensor=ug_ap.tensor, offset=ug_ap.offset + h * C,
                                ap=[list(ug_ap.ap[0]), [HC, nch],
                                    [2, C // 2], [1, 2]],
                            ),
                            in1=bass.AP(
                                tensor=a2_ap.tensor, offset=a2_ap.offset + h * 2,
                                ap=[list(a2_ap.ap[0]), [2 * H, nch],
                                    [0, C // 2], [1, 2]],
                            ),
                        )

                    agg_ps = psa.tile([P, H + HC], F32, tag="agg", space="PSUM")
                    for j in range(nch):
                        nc.tensor.matmul(
                            out=agg_ps[:], lhsT=mk[:, nch + j, :],
                            rhs=wcat[:, j, :],
                            start=(j == 0), stop=(j == nch - 1),
                        )

                    # epilogue: h_blk = mean_h(agg/den) (conv_bias folded
                    # into the GraphNorm affine)
                    den4 = sm.tile([P, H], F32, tag="d4")
                    nc.scalar.activation(
                        den4[:], agg_ps[:, :H], AF.Copy, scale=float(H),
                        bias=1e-12,
                    )
                    rec4 = sm.tile([P, H], F32, tag="rc")
                    nc.vector.reciprocal(out=rec4[:], in_=den4[:])
                    sc = sm.tile([P, HC], F32, tag="sc")
                    nc.vector.tensor_mul(
                        out=sc[:].rearrange("p (h c) -> p h c", h=H),
                        in0=agg_ps[:, H:].rearrange("p (h c) -> p h c", h=H),
                        in1=rec4[:].to_broadcast([P, H, C]),
                    )
                    nc.vector.tensor_reduce(
                        out=h_big[:, b, :],
                        in_=_ap3(sc[:], [1, C], [C, H]),
                        axis=mybir.AxisListType.X,
                        op=ALU.add,
                    )
                # ================= GraphNorm =================
                # stats (column sums of h and h^2) once per layer, off the
                # per-block critical path: PE chains over h_big / hsq
                hsq = med.tile([P, NBLK, C], F32, tag="gn")
                nc.vector.tensor_mul(out=hsq[:], in0=h_big[:], in1=h_big[:])
                for b in range(NBLK):
                    nc.tensor.matmul(
                        out=stats_ps[:C, :], lhsT=h_big[:, b, :], rhs=ones_col[:],
                        start=(b == 0), stop=(b == NBLK - 1),
                    )
                for b in range(NBLK):
                    nc.tensor.matmul(
                        out=stats_ps[C:, :], lhsT=hsq[:, b, :], rhs=ones_col[:],
                        start=(b == 0), stop=(b == NBLK - 1),
                    )
                stats_sb = sm.tile([P, 1], F32, tag="stsb")
                nc.scalar.activation(stats_sb[:], stats_ps[:], AF.Copy)
                nc.sync.dma_start(out=arin[l][:, None], in_=stats_sb[:])
                nc.gpsimd.collective_compute(
                    "AllReduce", ALU.add,
                    ins=[arin[l].opt()], outs=[arout[l].opt()],
                    replica_groups=[list(range(NC))],
                )
                srow = sm.tile([P, P], F32, tag="srow")
                nc.sync.dma_start(
                    out=srow[:],
                    in_=bass.AP(
                        tensor=arout[l].tensor, offset=arout[l].offset,
                        ap=[[0, P], [1, P]],
                    ),
                )
                g = gnc_bc[l]
                invN = 1.0 / float(N)
                m1 = sm.tile([P, C], F32, tag="m1")
                nc.scalar.activation(m1[:], srow[:, 0:C], AF.Copy, scale=invN)
                m2 = sm.tile([P, C], F32, tag="m2")
                nc.scalar.activation(m2[:], srow[:, C : 2 * C], AF.Copy, scale=invN)
                mu = sm.tile([P, C], F32, tag="mu")
                nc.vector.tensor_add(out=mu[:], in0=m1[:], in1=g[:, 0, :])
                t1 = sm.tile([P, C], F32, tag="t1")
                nc.vector.tensor_mul(out=t1[:], in0=mu[:], in1=mu[:])
                t2 = sm.tile([P, C], F32, tag="t2")
                nc.vector.tensor_mul(out=t2[:], in0=t1[:], in1=g[:, 3, :])
                u1 = sm.tile([P, C], F32, tag="u1")
                nc.vector.tensor_mul(out=u1[:], in0=m1[:], in1=g[:, 1, :])
                eh2 = sm.tile([P, C], F32, tag="eh2")
                nc.vector.tensor_add(out=eh2[:], in0=m2[:], in1=u1[:])
                nc.vector.tensor_add(out=eh2[:], in0=eh2[:], in1=g[:, 2, :])
                var = sm.tile([P, C], F32, tag="var")
                nc.vector.tensor_tensor(
                    out=var[:], in0=eh2[:], in1=t2[:], op=ALU.subtract
                )
                srt = sm.tile([P, C], F32, tag="srt")
                nc.scalar.activation(srt[:], var[:], AF.Sqrt, bias=eps_col[:])
                rst = sm.tile([P, C], F32, tag="rst")
                nc.vector.reciprocal(out=rst[:], in_=srt[:])
                A = sm.tile([P, C], F32, tag="A")
                nc.vector.tensor_mul(out=A[:], in0=rst[:], in1=g[:, 5, :])
                q = sm.tile([P, C], F32, tag="q")
                nc.vector.tensor_mul(out=q[:], in0=mu[:], in1=g[:, 4, :])
                nc.vector.tensor_tensor(
                    out=q[:], in0=g[:, 0, :], in1=q[:], op=ALU.subtract
                )
                Bt = sm.tile([P, C], F32, tag="B")
                nc.vector.tensor_mul(out=Bt[:], in0=A[:], in1=q[:])
                nc.vector.tensor_add(out=Bt[:], in0=Bt[:], in1=g[:, 6, :])

                # batched affine over all blocks, then per-block out/transpose
                xball = med.tile([P, NBLK, C], F32, tag="gn")
                nc.vector.tensor_mul(
                    out=xball[:], in0=h_big[:],
                    in1=_ap3(A[:], [0, NBLK], [1, C]),
                )
                nc.vector.tensor_add(
                    out=xball[:], in0=xball[:],
                    in1=_ap3(Bt[:], [0, NBLK], [1, C]),
                )
                for b in range(NBLK):
                    cnt = min(P, NSH - b * P)
                    if l == L - 1:
                        nc.sync.dma_start(
                            out=out[b * P : b * P + cnt, :],
                            in_=xball[:cnt, b, :],
                        )
                    else:
                        tp = ps.tile([D, P], F32, tag="pj", space="PSUM")
                        nc.tensor.transpose(
                            out=tp[:], in_=xball[:, b, :], identity=ident[:]
                        )
                        nc.scalar.activation(
                            xtsh_sb[:, b * P : b * P + P], tp[:], AF.Copy
                        )
                if l < L - 1:
                    nc.sync.dma_start(out=agin[l][:, :], in_=xtsh_sb[:, :NSH])
                    nc.gpsimd.collective_compute(
                        "AllGather", ALU.bypass,
                        ins=[agin[l].opt()], outs=[agout[l].opt()],
                        replica_groups=[list(range(NC))],
                    )
                    for r in range(NC):
                        hr, ir = r // 4, r % 4
                        nc.gpsimd.dma_start(
                            out=xt_pack[
                                hr * D : hr * D + D, ir * RW : ir * RW + NSH
                            ],
                            in_=agout[l][r, :, :],
                        )
                    nc.vector.tensor_copy(out=xtme_sb[:], in_=xtsh_sb[:])

    nc.compile()
    return nc


_CACHE = {}


def kernel(**inputs):
    cfg, in_maps = preprocess(inputs)
    key = (cfg["N"], tuple(cfg["nchunk"]))
    if key not in _CACHE:
        _CACHE[key] = build(cfg)
    nc = _CACHE[key]
    res = run_bass_kernel_spmd(nc, in_maps, core_ids=list(range(NC)))
    shards = [res.results[c]["out"] for c in range(NC)]
    return np.concatenate(shards, axis=0).astype(np.float32)


def _install_ntff_hook():
    import sys, types
    try:
        from antenv.axon_hooks import get_axon_ntff_profile_hook  # noqa
        return
    except ImportError:
        pass
    import trn_agent_boot.trn_boot as tb
    mod = types.ModuleType("antenv.axon_hooks")
    _hook = [None]
    mod.set_axon_ntff_profile_hook = lambda h: _hook.__setitem__(0, h)
    mod.get_axon_ntff_profile_hook = lambda: _hook[0]
    sys.modules["antenv.axon_hooks"] = mod
    import antenv
    antenv.axon_hooks = mod
    mod.set_axon_ntff_profile_hook(
        tb._ntff_profile_via_ctypes("/opt/axon/libaxon_pjrt.so")
    )


def run_traced(**inputs):
    """Re-run the cached kernel with NTFF tracing; returns exec_time_ns."""
    _install_ntff_hook()
    cfg, in_maps = preprocess(inputs)
    key = (cfg["N"], tuple(cfg["nchunk"]))
    if key not in _CACHE:
        _CACHE[key] = build(cfg)
    nc = _CACHE[key]
    res = run_bass_kernel_spmd(
        nc, in_maps, core_ids=list(range(NC)), trace=True
    )
    return res.exec_time_ns


# revision 17
# speedup vs baseline: 1.0961x; 1.0961x over previous
"""GATv2 (3-layer, 4-head, GraphNorm) Bass kernel for 8 trn2 NeuronCores.

Sharding: nodes partitioned by dst across 8 cores. Each core writes a
combined projection table (xl for all shards + xr for its own shard) to
DRAM with batched DMAs, gathers xl[src] and xr[dst] per 128-dst-node block
via SWDGE dma_gather, sums them with identity matmuls in PSUM, does
block-batched edge math with host-precomputed one-hot dst masks feeding
the segment-softmax aggregation matmuls, then GraphNorm with an AllReduce
for global stats and an AllGather of transposed node-feature shards
feeding the next layer's projections.
"""
import math

import ml_dtypes
import numpy as np

import concourse.bacc as bacc
import concourse.bass as bass
import concourse.tile as tile
from concourse import mybir
from concourse.bass_utils import run_bass_kernel_spmd
from concourse.masks import make_identity

F32 = mybir.dt.float32
BF16 = mybir.dt.bfloat16
I16 = mybir.dt.int16
I32 = mybir.dt.int32
AF = mybir.ActivationFunctionType
ALU = mybir.AluOpType

NC = 8
D = 64
H = 4
C = 64
HC = H * C  # 256
L = 3
NEG = 0.2
EPS = 1e-5
P = 128
NQ = 4  # SWDGE queues (1-4)


def _bf(x):
    return np.asarray(x, dtype=ml_dtypes.bfloat16)


def _wrap_idx(idx):
    """[n*128] int -> [128, n*8] int16 wrapped in 16 partitions, replicated
    across the 8 gpsimd core groups."""
    n = idx.shape[0]
    assert n % 128 == 0
    w = idx.reshape(n // 16, 16).T  # [16, n//16]
    return np.tile(w, (8, 1)).astype(np.int16)


def preprocess(inputs):
    """Host-side: shard/sort/pad edges, build all per-core input tensors."""
    x = np.asarray(inputs["x"], np.float32)
    ei = np.asarray(inputs["edge_index"], np.int64)
    Wl = np.asarray(inputs["Wl"], np.float32)
    bl = np.asarray(inputs["bl"], np.float32)
    Wr = np.asarray(inputs["Wr"], np.float32)
    br = np.asarray(inputs["br"], np.float32)
    att = np.asarray(inputs["att"], np.float32)
    conv_bias = np.asarray(inputs["conv_bias"], np.float32)
    gn_weight = np.asarray(inputs["gn_weight"], np.float32)
    gn_scale = np.asarray(inputs["gn_scale"], np.float32)
    gn_bias = np.asarray(inputs["gn_bias"], np.float32)

    N = x.shape[0]
    NSH = N // NC
    NBLK = (NSH + P - 1) // P
    RW = NBLK * P
    NT = N + (-N) % P

    loop = np.arange(N, dtype=np.int64)
    src = np.concatenate([ei[0], loop])
    dst = np.concatenate([ei[1], loop])

    per_core = []
    cnts = np.zeros((NC, NBLK), np.int64)
    for c in range(NC):
        sel = (dst >= c * NSH) & (dst < (c + 1) * NSH)
        s = src[sel].astype(np.int32)
        dl = (dst[sel] - c * NSH).astype(np.int32)
        order = np.argsort(dl, kind="stable")
        s, dl = s[order], dl[order]
        blk = dl // P
        starts = np.searchsorted(blk, np.arange(NBLK))
        ends = np.searchsorted(blk, np.arange(NBLK), side="right")
        cnts[c] = ends - starts
        per_core.append((s, dl, starts, ends))

    nchunk = [max(1, int(math.ceil(cnts[:, b].max() / P))) for b in range(NBLK)]
    IWC = int(sum(nchunk))
    cum = np.concatenate([[0], np.cumsum(nchunk)]).astype(int)

    iota128 = np.arange(P, dtype=np.int32)
    ONE_BF = np.uint16(0x3F80)  # 1.0 in bf16 bits

    in_maps = []
    for c in range(NC):
        s, dl, starts, ends = per_core[c]
        srcw = np.zeros((P, IWC * 8), np.int16)
        # per block: [s mask (nch*128 cols) | st mask (nch*128 cols)]
        # s[i, k, e] = (dloc[k,e] == i) with partitions=i (dst-local row);
        # st[e, k, i] = same predicate with partitions=e (edge lane)
        stm = np.zeros((P, IWC * 2 * P), np.uint16)
        for b in range(NBLK):
            ns = nchunk[b] * P
            e0, e1 = starts[b], ends[b]
            n = e1 - e0
            sp = np.zeros(ns, np.int16)
            sp[:n] = s[e0:e1]
            lp = np.full(ns, -1, np.int32)  # pad lane: matches no i
            lp[:n] = dl[e0:e1] - b * P
            co = int(cum[b]) * 8
            srcw[:, co : co + nchunk[b] * 8] = _wrap_idx(sp)
            lp_r = lp.reshape(nchunk[b], P)
            m_s = (lp_r[None, :, :] == iota128[:, None, None])  # [i, k, e]
            mo = int(cum[b]) * 2 * P
            stm[:, mo : mo + ns] = m_s.reshape(P, ns) * ONE_BF
            stm[:, mo + ns : mo + 2 * ns] = (
                m_s.transpose(2, 1, 0).reshape(P, ns) * ONE_BF
            )
        in_maps.append(
            {"srcw": srcw, "stm": stm.view(ml_dtypes.bfloat16)}
        )

    # xl carries no bias: bl is folded into the xr-side bias for the logits
    # path (xl+xr unchanged) and into the GraphNorm affine for the
    # aggregation path (mean over heads of bl is a constant shift of h).
    wts = np.zeros((L, 2, D + 1, HC), np.float32)
    for l in range(L):
        wts[l, 0, :D] = Wl[l].T
        wts[l, 1, :D] = Wr[l].T
        wts[l, 1, D] = bl[l] + br[l]
    wts = _bf(wts)

    attb = _bf(att.reshape(L, HC))

    cb_eff = conv_bias + bl.reshape(L, H, C).mean(axis=1)
    gnc = np.stack(
        [
            cb_eff,
            2 * cb_eff,
            cb_eff * cb_eff,
            gn_scale * (2 - gn_scale),
            gn_scale,
            gn_weight,
            gn_bias,
        ],
        axis=1,
    ).astype(np.float32)  # [L, 7, C]

    xt0 = np.zeros((P, 4 * RW), np.float32)
    for r in range(NC):
        hr, ir = r // 4, r % 4
        xt0[hr * D : hr * D + D, ir * RW : ir * RW + NSH] = x[
            r * NSH : (r + 1) * NSH
        ].T
    xt0 = _bf(xt0)

    for c in range(NC):
        in_maps[c]["wts"] = wts
        in_maps[c]["attb"] = attb
        in_maps[c]["gnc"] = gnc
        in_maps[c]["xt0"] = xt0
        xtme = np.zeros((D, RW), np.float32)
        xtme[:, :NSH] = x[c * NSH : (c + 1) * NSH].T
        in_maps[c]["xtme0"] = _bf(xtme)

    cfg = dict(
        N=N, NSH=NSH, NBLK=NBLK, RW=RW, nchunk=[int(v) for v in nchunk],
        cum=[int(v) for v in cum], IWC=IWC,
    )
    return cfg, in_maps


def _ap3(ap, d1, d2):
    """Build [P, d1, d2] AP from a 2D AP by appending explicit dims."""
    return bass.AP(tensor=ap.tensor, offset=ap.offset, ap=[list(ap.ap[0]), d1, d2])


def build(cfg):
    N, NSH, NBLK, RW = cfg["N"], cfg["NSH"], cfg["NBLK"], cfg["RW"]
    nchunk, cum, IWC = cfg["nchunk"], cfg["cum"], cfg["IWC"]
    NT = N + (-N) % P
    nRT = (NSH + P - 1) // P
    NFULL = NSH // P  # full 128-row tiles per shard
    NREM = NSH - NFULL * P  # rows in the last partial tile

    nc = bacc.Bacc(
        "TRN2",
        target_bir_lowering=False,
        debug=False,
        num_devices=NC,
        num_swdge_queues=NQ,
        dynamic_dma_scratch_size=32768,
    )

    srcw = nc.dram_tensor("srcw", [P, IWC * 8], I16, kind="ExternalInput").ap()
    stm = nc.dram_tensor("stm", [P, IWC * 2 * P], BF16, kind="ExternalInput").ap()
    wts = nc.dram_tensor("wts", [L, 2, D + 1, HC], BF16, kind="ExternalInput").ap()
    attb = nc.dram_tensor("attb", [L, HC], BF16, kind="ExternalInput").ap()
    gnc = nc.dram_tensor("gnc", [L, 7, C], F32, kind="ExternalInput").ap()
    xt0 = nc.dram_tensor("xt0", [P, 4 * RW], BF16, kind="ExternalInput").ap()
    xtme0 = nc.dram_tensor("xtme0", [D, RW], BF16, kind="ExternalInput").ap()
    out = nc.dram_tensor("out", [NSH, C], F32, kind="ExternalOutput").ap()

    tab = nc.dram_tensor("tab", [NT, HC], BF16).ap()
    arin = [nc.dram_tensor(f"arin{l}", [P], F32).ap() for l in range(L)]
    arout = [
        nc.dram_tensor(f"arout{l}", [P], F32, addr_space="Shared").ap()
        for l in range(L)
    ]
    agin = [nc.dram_tensor(f"agin{l}", [D, NSH], BF16).ap() for l in range(L - 1)]
    agout = [
        nc.dram_tensor(f"agout{l}", [NC, D, NSH], BF16, addr_space="Shared").ap()
        for l in range(L - 1)
    ]

    def _tab_out_ap(row0, pcnt, nb):
        """DRAM AP over tab rows row0 + b*128 + p (p outer, then b, then c)."""
        return bass.AP(
            tensor=tab.tensor,
            offset=tab.offset + row0 * HC,
            ap=[[HC, pcnt], [P * HC, nb], [1, HC]],
        )

    with tile.TileContext(nc) as tc:
        with (
            tc.tile_pool(name="res", bufs=1) as res,
            tc.tile_pool(name="stg", bufs=2) as stg,
            tc.tile_pool(name="big", bufs=2) as big,
            tc.tile_pool(name="gat", bufs=2) as gat,
            tc.tile_pool(name="idx", bufs=3) as idxp,
            tc.tile_pool(name="msk", bufs=2) as msk,
            tc.tile_pool(name="med", bufs=1) as med,
            tc.tile_pool(name="sm", bufs=2) as sm,
            tc.tile_pool(name="ps", bufs=2, space="PSUM") as ps,
            tc.tile_pool(name="psa", bufs=2, space="PSUM") as psa,
            tc.tile_pool(name="psb", bufs=1, space="PSUM") as psb,
            tc.tile_pool(name="psx", bufs=3, space="PSUM") as psx,
        ):
            # ---- resident loads / constants ----

            ones_row = res.tile([1, P], BF16)
            nc.vector.memset(ones_row[:], 1.0)
            ones_col = res.tile([P, 1], F32)
            nc.vector.memset(ones_col[:], 1.0)
            ident = res.tile([P, P], F32)
            make_identity(nc, ident[:])
            ident_bf = res.tile([P, P], BF16)
            nc.vector.tensor_copy(out=ident_bf[:], in_=ident[:])
            eps_col = res.tile([P, 1], F32)
            nc.vector.memset(eps_col[:], EPS)

            xt_pack = res.tile([P, 4 * RW], BF16)
            nc.sync.dma_start(out=xt_pack[:], in_=xt0[:, :])
            xtme_sb = res.tile([D, RW], BF16)
            nc.sync.dma_start(out=xtme_sb[:], in_=xtme0[:, :])

            w_tiles = {}
            b_tiles = {}
            for l in range(L):
                for side in range(2):
                    # weights duplicated into both partition halves so lhsT
                    # slices based at partition 0 or 64 both find a matching
                    # rhs base
                    t = res.tile([P, HC], BF16, tag=f"w{l}{side}")
                    nc.sync.dma_start(out=t[:D, :], in_=wts[l, side, :D, :])
                    nc.sync.dma_start(out=t[D:, :], in_=wts[l, side, :D, :])
                    w_tiles[(l, side)] = t
                    if side == 1:
                        bt = res.tile([1, HC], BF16, tag=f"b{l}{side}")
                        nc.sync.dma_start(
                            out=bt[:], in_=wts[l, side, D : D + 1, :]
                        )
                        b_tiles[(l, side)] = bt

            att_bc = {}
            for l in range(L):
                t = res.tile([P, HC], BF16, tag=f"att{l}")
                nc.sync.dma_start(
                    out=t[:],
                    in_=bass.AP(
                        tensor=attb.tensor, offset=attb.offset + l * HC,
                        ap=[[0, P], [1, HC]],
                    ),
                )
                att_bc[l] = t

            gnc_bc = {}
            for l in range(L):
                t = res.tile([P, 7, C], F32, tag=f"gnc{l}")
                nc.sync.dma_start(
                    out=t[:],
                    in_=bass.AP(
                        tensor=gnc.tensor, offset=gnc.offset + l * 7 * C,
                        ap=[[0, P], [C, 7], [1, C]],
                    ),
                )
                gnc_bc[l] = t

            xr_res = res.tile([P, NBLK, HC], BF16)
            h_big = res.tile([P, NBLK, C], F32)
            xtsh_sb = res.tile([D, RW], BF16)
            nc.vector.memset(xtsh_sb[:], 0.0)

            for l in range(L):
                # ================= projections =================
                # xr first: it only needs own-shard features (xtme_sb), so on
                # layer boundaries PE runs it while the AllGather that feeds
                # xt_pack (needed by the xl loop below) is still in flight.
                for j in range(nRT):
                    n0 = j * P
                    lhsT = xtme_sb[:, n0 : n0 + P]
                    pt = ps.tile([P, HC], F32, tag="pj", space="PSUM")
                    nc.tensor.matmul(
                        out=pt[:], lhsT=lhsT, rhs=w_tiles[(l, 1)][:D, :],
                        start=True, stop=False,
                    )
                    nc.tensor.matmul(
                        out=pt[:], lhsT=ones_row[:],
                        rhs=b_tiles[(l, 1)][:],
                        start=False, stop=True,
                    )
                    if j % 2 == 0:
                        nc.scalar.activation(xr_res[:, j, :], pt[:], AF.Copy)
                    else:
                        nc.vector.tensor_copy(out=xr_res[:, j, :], in_=pt[:])

                for r in range(NC):
                    hr, ir = r // 4, r % 4
                    xl_st = stg.tile([P, nRT, HC], BF16, tag="pst")
                    for j in range(nRT):
                        n0 = j * P
                        lhsT = xt_pack[
                            hr * D : hr * D + D, ir * RW + n0 : ir * RW + n0 + P
                        ]
                        pt = ps.tile([P, HC], F32, tag="pj", space="PSUM")
                        nc.tensor.matmul(
                            out=pt[:], lhsT=lhsT,
                            rhs=w_tiles[(l, 0)][hr * D : hr * D + D, :],
                            start=True, stop=True,
                        )
                        if j % 2 == 0:
                            nc.scalar.activation(xl_st[:, j, :], pt[:], AF.Copy)
                        else:
                            nc.vector.tensor_copy(out=xl_st[:, j, :], in_=pt[:])
                    g0 = r * NSH
                    nc.sync.dma_start(
                        out=_tab_out_ap(g0, P, NFULL // 2),
                        in_=xl_st[:, : NFULL // 2, :],
                    )
                    nc.sync.dma_start(
                        out=_tab_out_ap(g0 + (NFULL // 2) * P, P, NFULL - NFULL // 2),
                        in_=xl_st[:, NFULL // 2 : NFULL, :],
                    )
                    if NREM:
                        nc.sync.dma_start(
                            out=_tab_out_ap(g0 + NFULL * P, NREM, 1),
                            in_=xl_st[:NREM, NFULL, :],
                        )

                # ================= edge blocks =================
                stats_ps = psb.tile([P, 1], F32, tag="stats", space="PSUM")
                gq = 0
                for b in range(NBLK):
                    nch = nchunk[b]
                    co = cum[b]

                    # gather xl[src] (chunks 0..nch) and xr[dst] (nch..2nch)
                    # dma_gather tops out at 1024 indices -- split into
                    # sub-calls (xl/xr interleaved per 8 chunks, see
                    # preprocess), round-robin across the 4 SWDGE queues
                    xl_g = gat.tile([P, nch, HC], BF16, tag="ug")
                    idx_sb = idxp.tile([P, nch * 8], I16, tag="ix")
                    nc.sync.dma_start(
                        out=idx_sb[:],
                        in_=srcw[:, co * 8 : (co + nch) * 8],
                    )
                    for g in range(0, nch, 8):
                        gn = min(8, nch - g)
                        sub = gn * P
                        nc.gpsimd.dma_gather(
                            out_ap=xl_g[:, g : g + gn, :], in_ap=tab[:, :],
                            idxs_ap=idx_sb[:, g * 8 : (g + gn) * 8],
                            num_idxs=sub, num_idxs_reg=sub, elem_size=HC,
                            queue_num=gq % NQ,
                        )
                        gq += 1

                    # host-precomputed one-hot dst masks: s[i,k,e] | st[e,k,i]
                    mk = msk.tile([P, 2 * nch, P], BF16, tag="mk")
                    mo = co * 2 * P
                    nc.sync.dma_start(
                        out=mk[:, :nch, :],
                        in_=stm[:, mo : mo + nch * P],
                    )
                    nc.sync.dma_start(
                        out=mk[:, nch:, :],
                        in_=stm[:, mo + nch * P : mo + 2 * nch * P],
                    )

                    # u = xl[src] + xr[dst] accumulated in PSUM via identity
                    # matmuls, then leaky-relu straight from PSUM to SBUF
                    lr = big.tile([P, nch, HC], BF16, tag="g1")
                    for j in range(0, nch, 2):
                        jn = min(2, nch - j)
                        ups = psx.tile([P, 2, HC], F32, tag="xre", space="PSUM")
                        # one wide identity matmul folds xl for both chunks,
                        # then per-chunk mask matmuls add the xr expansion
                        nc.tensor.matmul(
                            out=ups[:, :jn, :].rearrange("p a b -> p (a b)"),
                            lhsT=ident_bf[:],
                            rhs=xl_g[:, j : j + jn, :].rearrange(
                                "p a b -> p (a b)"
                            ),
                            start=True, stop=False,
                        )
                        for t in range(jn):
                            nc.tensor.matmul(
                                out=ups[:, t, :], lhsT=mk[:, j + t, :],
                                rhs=xr_res[:, b, :],
                                start=False, stop=(t == jn - 1),
                            )
                        nc.scalar.activation(
                            lr[:, j : j + jn, :], ups[:, :jn, :],
                            AF.Prelu, alpha=NEG,
                        )

                    v = big.tile([P, nch, HC], BF16, tag="g2")
                    ab = att_bc[l][:]
                    nc.vector.tensor_mul(
                        out=v[:], in0=lr[:], in1=_ap3(ab, [0, nch], [1, HC])
                    )
                    logits = sm.tile([P, nch, H], F32, tag="lg")
                    # tree-reduce over C: 64 -> 32 -> 16 via 2x-mode TT adds,
                    # then one 1x tensor_reduce over the remaining 16
                    # r1/r2 scratch lives in lr, which is dead after the
                    # v = lr*att multiply above
                    r1 = lr[:, :, 0:128].rearrange("p n (h c) -> p n h c", h=H)
                    r2 = lr[:, :, 128:192].rearrange(
                        "p n (h c) -> p n h c", h=H
                    )
                    vv = v[:].rearrange("p n (h c) -> p n h c", h=H)
                    nc.vector.tensor_tensor(
                        out=r1, in0=vv[:, :, :, :32], in1=vv[:, :, :, 32:],
                        op=ALU.add,
                    )
                    nc.vector.tensor_tensor(
                        out=r2, in0=r1[:, :, :, :16], in1=r1[:, :, :, 16:],
                        op=ALU.add,
                    )
                    nc.vector.tensor_reduce(
                        out=logits[:],
                        in_=r2,
                        axis=mybir.AxisListType.X,
                        op=ALU.add,
                    )
                    # wcat: cols 0:H hold a=exp(logits), cols H: hold a*xl
                    wcat = big.tile([P, nch, H + HC], BF16, tag="g2")
                    nc.scalar.activation(wcat[:, :, :H], logits[:], AF.Exp)
                    # a2: each a value duplicated twice (contiguous) so the
                    # per-head multiply below reads it with an inner [1,2]
                    # stride pattern -> DVE 2x mode
                    a2 = sm.tile([P, nch, H, 2], BF16, tag="a2")
                    lg_ap = logits[:]
                    nc.scalar.activation(
                        a2[:],
                        bass.AP(
                            tensor=lg_ap.tensor, offset=lg_ap.offset,
                            ap=[list(lg_ap.ap[0]), [H, nch], [1, H], [0, 2]],
                        ),
                        AF.Exp,
                    )
                    wc_ap = wcat[:]
                    ug_ap = xl_g[:]
                    a2_ap = a2[:]
                    for h in range(H):
                        nc.vector.tensor_mul(
                            out=bass.AP(
                                tensor=wc_ap.tensor,
                                offset=wc_ap.offset + H + h * C,
                                ap=[list(wc_ap.ap[0]), [H + HC, nch],
                                    [2, C // 2], [1, 2]],
                            ),
                            in0=bass.AP(
                                tensor=ug_ap.tensor, offset=ug_ap.offset + h * C,
                                ap=[list(ug_ap.ap[0]), [HC, nch],
                                    [2, C // 2], [1, 2]],
                            ),
                            in1=bass.AP(
                                tensor=a2_ap.tensor, offset=a2_ap.offset + h * 2,
                                ap=[list(a2_ap.ap[0]), [2 * H, nch],
                                    [0, C // 2], [1, 2]],
                            ),
                        )

                    agg_ps = psa.tile([P, H + HC], F32, tag="agg", space="PSUM")
                    for j in range(nch):
                        nc.tensor.matmul(
                            out=agg_ps[:], lhsT=mk[:, nch + j, :],
                            rhs=wcat[:, j, :],
                            start=(j == 0), stop=(j == nch - 1),
                        )

                    # epilogue: h_blk = mean_h(agg/den) (conv_bias folded
                    # into the GraphNorm affine)
                    den4 = sm.tile([P, H], F32, tag="d4")
                    nc.scalar.activation(
                        den4[:], agg_ps[:, :H], AF.Copy, scale=float(H),
                        bias=1e-12,
                    )
                    rec4 = sm.tile([P, H], F32, tag="rc")
                    nc.vector.reciprocal(out=rec4[:], in_=den4[:])
                    sc = sm.tile([P, HC], F32, tag="sc")
                    nc.vector.tensor_mul(
                        out=sc[:].rearrange("p (h c) -> p h c", h=H),
                        in0=agg_ps[:, H:].rearrange("p (h c) -> p h c", h=H),
                        in1=rec4[:].to_broadcast([P, H, C]),
                    )
                    nc.vector.tensor_reduce(
                        out=h_big[:, b, :],
                        in_=_ap3(sc[:], [1, C], [C, H]),
                        axis=mybir.AxisListType.X,
                        op=ALU.add,
                    )
                # ================= GraphNorm =================
                # stats (column sums of h and h^2) once per layer, off the
                # per-block critical path: PE chains over h_big / hsq
                hsq = med.tile([P, NBLK, C], F32, tag="gn")
                nc.vector.tensor_mul(out=hsq[:], in0=h_big[:], in1=h_big[:])
                for b in range(NBLK):
                    nc.tensor.matmul(
                        out=stats_ps[:C, :], lhsT=h_big[:, b, :], rhs=ones_col[:],
                        start=(b == 0), stop=(b == NBLK - 1),
                    )
                for b in range(NBLK):
                    nc.tensor.matmul(
                        out=stats_ps[C:, :], lhsT=hsq[:, b, :], rhs=ones_col[:],
                        start=(b == 0), stop=(b == NBLK - 1),
                    )
                stats_sb = sm.tile([P, 1], F32, tag="stsb")
                nc.scalar.activation(stats_sb[:], stats_ps[:], AF.Copy)
                nc.sync.dma_start(out=arin[l][:, None], in_=stats_sb[:])
                nc.gpsimd.collective_compute(
                    "AllReduce", ALU.add,
                    ins=[arin[l].opt()], outs=[arout[l].opt()],
                    replica_groups=[list(range(NC))],
                )
                srow = sm.tile([P, P], F32, tag="srow")
                nc.sync.dma_start(
                    out=srow[:],
                    in_=bass.AP(
                        tensor=arout[l].tensor, offset=arout[l].offset,
                        ap=[[0, P], [1, P]],
                    ),
                )
                g = gnc_bc[l]
                invN = 1.0 / float(N)
                m1 = sm.tile([P, C], F32, tag="m1")
                nc.scalar.activation(m1[:], srow[:, 0:C], AF.Copy, scale=invN)
                m2 = sm.tile([P, C], F32, tag="m2")
                nc.scalar.activation(m2[:], srow[:, C : 2 * C], AF.Copy, scale=invN)
                mu = sm.tile([P, C], F32, tag="mu")
                nc.vector.tensor_add(out=mu[:], in0=m1[:], in1=g[:, 0, :])
                t1 = sm.tile([P, C], F32, tag="t1")
                nc.vector.tensor_mul(out=t1[:], in0=mu[:], in1=mu[:])
                t2 = sm.tile([P, C], F32, tag="t2")
                nc.vector.tensor_mul(out=t2[:], in0=t1[:], in1=g[:, 3, :])
                u1 = sm.tile([P, C], F32, tag="u1")
                nc.vector.tensor_mul(out=u1[:], in0=m1[:], in1=g[:, 1, :])
                eh2 = sm.tile([P, C], F32, tag="eh2")
                nc.vector.tensor_add(out=eh2[:], in0=m2[:], in1=u1[:])
                nc.vector.tensor_add(out=eh2[:], in0=eh2[:], in1=g[:, 2, :])
                var = sm.tile([P, C], F32, tag="var")
                nc.vector.tensor_tensor(
                    out=var[:], in0=eh2[:], in1=t2[:], op=ALU.subtract
                )
                srt = sm.tile([P, C], F32, tag="srt")
                nc.scalar.activation(srt[:], var[:], AF.Sqrt, bias=eps_col[:])
                rst = sm.tile([P, C], F32, tag="rst")
                nc.vector.reciprocal(out=rst[:], in_=srt[:])
                A = sm.tile([P, C], F32, tag="A")
                nc.vector.tensor_mul(out=A[:], in0=rst[:], in1=g[:, 5, :])
                q = sm.tile([P, C], F32, tag="q")
                nc.vector.tensor_mul(out=q[:], in0=mu[:], in1=g[:, 4, :])
                nc.vector.tensor_tensor(
                    out=q[:], in0=g[:, 0, :], in1=q[:], op=ALU.subtract
                )
                Bt = sm.tile([P, C], F32, tag="B")
                nc.vector.tensor_mul(out=Bt[:], in0=A[:], in1=q[:])
                nc.vector.tensor_add(out=Bt[:], in0=Bt[:], in1=g[:, 6, :])

                # batched affine over all blocks, then per-block out/transpose
                xball = med.tile([P, NBLK, C], F32, tag="gn")
                nc.vector.tensor_mul(
                    out=xball[:], in0=h_big[:],
                    in1=_ap3(A[:], [0, NBLK], [1, C]),
                )
                nc.vector.tensor_add(
                    out=xball[:], in0=xball[:],
                    in1=_ap3(Bt[:], [0, NBLK], [1, C]),
                )
                for b in range(NBLK):
                    cnt = min(P, NSH - b * P)
                    if l == L - 1:
                        nc.sync.dma_start(
                            out=out[b * P : b * P + cnt, :],
                            in_=xball[:cnt, b, :],
                        )
                    else:
                        tp = ps.tile([D, P], F32, tag="pj", space="PSUM")
                        nc.tensor.transpose(
                            out=tp[:], in_=xball[:, b, :], identity=ident[:]
                        )
                        nc.scalar.activation(
                            xtsh_sb[:, b * P : b * P + P], tp[:], AF.Copy
                        )
                if l < L - 1:
                    nc.sync.dma_start(out=agin[l][:, :], in_=xtsh_sb[:, :NSH])
                    nc.gpsimd.collective_compute(
                        "AllGather", ALU.bypass,
                        ins=[agin[l].opt()], outs=[agout[l].opt()],
                        replica_groups=[list(range(NC))],
                    )
                    for r in range(NC):
                        hr, ir = r // 4, r % 4
                        nc.gpsimd.dma_start(
                            out=xt_pack[
                                hr * D : hr * D + D, ir * RW : ir * RW + NSH
                            ],
                            in_=agout[l][r, :, :],
                        )
                    nc.vector.tensor_copy(out=xtme_sb[:], in_=xtsh_sb[:])

    nc.compile()
    return nc


_CACHE = {}


def kernel(**inputs):
    cfg, in_maps = preprocess(inputs)
    key = (cfg["N"], tuple(cfg["nchunk"]))
    if key not in _CACHE:
        _CACHE[key] = build(cfg)
    nc = _CACHE[key]
    res = run_bass_kernel_spmd(nc, in_maps, core_ids=list(range(NC)))
    shards = [res.results[c]["out"] for c in range(NC)]
    return np.concatenate(shards, axis=0).astype(np.float32)


def _install_ntff_hook():
    import sys, types
    try:
        from antenv.axon_hooks import get_axon_ntff_profile_hook  # noqa
        return
    except ImportError:
        pass
    import trn_agent_boot.trn_boot as tb
    mod = types.ModuleType("antenv.axon_hooks")
    _hook = [None]
    mod.set_axon_ntff_profile_hook = lambda h: _hook.__setitem__(0, h)
    mod.get_axon_ntff_profile_hook = lambda: _hook[0]
    sys.modules["antenv.axon_hooks"] = mod
    import antenv
    antenv.axon_hooks = mod
    mod.set_axon_ntff_profile_hook(
        tb._ntff_profile_via_ctypes("/opt/axon/libaxon_pjrt.so")
    )


def run_traced(**inputs):
    """Re-run the cached kernel with NTFF tracing; returns exec_time_ns."""
    _install_ntff_hook()
    cfg, in_maps = preprocess(inputs)
    key = (cfg["N"], tuple(cfg["nchunk"]))
    if key not in _CACHE:
        _CACHE[key] = build(cfg)
    nc = _CACHE[key]
    res = run_bass_kernel_spmd(
        nc, in_maps, core_ids=list(range(NC)), trace=True
    )
    return res.exec_time_ns


# revision 39
# speedup vs baseline: 1.2225x; 1.1153x over previous
"""GATv2 (3-layer, 4-head, GraphNorm) Bass kernel for 8 trn2 NeuronCores.

Sharding: nodes partitioned by dst across 8 cores. Each core writes a
combined projection table (xl for all shards + xr for its own shard) to
DRAM with batched DMAs, gathers xl[src] and xr[dst] per 128-dst-node block
via SWDGE dma_gather, sums them with identity matmuls in PSUM, does
block-batched edge math with host-precomputed one-hot dst masks feeding
the segment-softmax aggregation matmuls, then GraphNorm with an AllReduce
for global stats and an AllGather of transposed node-feature shards
feeding the next layer's projections.
"""
import math

import ml_dtypes
import numpy as np

import concourse.bacc as bacc
import concourse.bass as bass
import concourse.tile as tile
from concourse import mybir
from concourse.bass_utils import run_bass_kernel_spmd
from concourse.masks import make_identity

F32 = mybir.dt.float32
BF16 = mybir.dt.bfloat16
I16 = mybir.dt.int16
I32 = mybir.dt.int32
AF = mybir.ActivationFunctionType
ALU = mybir.AluOpType

NC = 8
D = 64
H = 4
C = 64
HC = H * C  # 256
L = 3
NEG = 0.2
EPS = 1e-5
P = 128
NQ = 4  # SWDGE queues (1-4)


def _bf(x):
    return np.asarray(x, dtype=ml_dtypes.bfloat16)


def _wrap_idx(idx):
    """[n*128] int -> [128, n*8] int16 wrapped in 16 partitions, replicated
    across the 8 gpsimd core groups."""
    n = idx.shape[0]
    assert n % 128 == 0
    w = idx.reshape(n // 16, 16).T  # [16, n//16]
    return np.tile(w, (8, 1)).astype(np.int16)


def preprocess(inputs):
    """Host-side: shard/sort/pad edges, build all per-core input tensors."""
    x = np.asarray(inputs["x"], np.float32)
    ei = np.asarray(inputs["edge_index"], np.int64)
    Wl = np.asarray(inputs["Wl"], np.float32)
    bl = np.asarray(inputs["bl"], np.float32)
    Wr = np.asarray(inputs["Wr"], np.float32)
    br = np.asarray(inputs["br"], np.float32)
    att = np.asarray(inputs["att"], np.float32)
    conv_bias = np.asarray(inputs["conv_bias"], np.float32)
    gn_weight = np.asarray(inputs["gn_weight"], np.float32)
    gn_scale = np.asarray(inputs["gn_scale"], np.float32)
    gn_bias = np.asarray(inputs["gn_bias"], np.float32)

    N = x.shape[0]
    NSH = N // NC
    NBLK = (NSH + P - 1) // P
    RW = NBLK * P
    NT = N + (-N) % P

    loop = np.arange(N, dtype=np.int64)
    src = np.concatenate([ei[0], loop])
    dst = np.concatenate([ei[1], loop])

    per_core = []
    cnts = np.zeros((NC, NBLK), np.int64)
    for c in range(NC):
        sel = (dst >= c * NSH) & (dst < (c + 1) * NSH)
        s = src[sel].astype(np.int32)
        dl = (dst[sel] - c * NSH).astype(np.int32)
        order = np.argsort(dl, kind="stable")
        s, dl = s[order], dl[order]
        blk = dl // P
        starts = np.searchsorted(blk, np.arange(NBLK))
        ends = np.searchsorted(blk, np.arange(NBLK), side="right")
        cnts[c] = ends - starts
        per_core.append((s, dl, starts, ends))

    nchunk = [max(1, int(math.ceil(cnts[:, b].max() / P))) for b in range(NBLK)]
    IWC = int(sum(nchunk))
    cum = np.concatenate([[0], np.cumsum(nchunk)]).astype(int)

    iota128 = np.arange(P, dtype=np.int32)
    ONE_BF = np.uint16(0x3F80)  # 1.0 in bf16 bits

    in_maps = []
    for c in range(NC):
        s, dl, starts, ends = per_core[c]
        srcw = np.zeros((P, IWC * 8), np.int16)
        # per block: [s mask (nch*128 cols) | st mask (nch*128 cols)]
        # s[i, k, e] = (dloc[k,e] == i) with partitions=i (dst-local row);
        # st[e, k, i] = same predicate with partitions=e (edge lane)
        stm = np.zeros((P, IWC * 2 * P), np.uint16)
        for b in range(NBLK):
            ns = nchunk[b] * P
            e0, e1 = starts[b], ends[b]
            n = e1 - e0
            sp = np.zeros(ns, np.int16)
            sp[:n] = s[e0:e1]
            lp = np.full(ns, -1, np.int32)  # pad lane: matches no i
            lp[:n] = dl[e0:e1] - b * P
            co = int(cum[b]) * 8
            srcw[:, co : co + nchunk[b] * 8] = _wrap_idx(sp)
            lp_r = lp.reshape(nchunk[b], P)
            m_s = (lp_r[None, :, :] == iota128[:, None, None])  # [i, k, e]
            mo = int(cum[b]) * 2 * P
            stm[:, mo : mo + ns] = m_s.reshape(P, ns) * ONE_BF
            stm[:, mo + ns : mo + 2 * ns] = (
                m_s.transpose(2, 1, 0).reshape(P, ns) * ONE_BF
            )
        in_maps.append(
            {"srcw": srcw, "stm": stm.view(ml_dtypes.bfloat16)}
        )

    # xl carries no bias: bl is folded into the xr-side bias for the logits
    # path (xl+xr unchanged) and into the GraphNorm affine for the
    # aggregation path (mean over heads of bl is a constant shift of h).
    wts = np.zeros((L, 2, D + 1, HC), np.float32)
    for l in range(L):
        wts[l, 0, :D] = Wl[l].T
        wts[l, 1, :D] = Wr[l].T
        wts[l, 1, D] = bl[l] + br[l]
    wts = _bf(wts)

    attb = _bf(att.reshape(L, HC))

    cb_eff = conv_bias + bl.reshape(L, H, C).mean(axis=1)
    gnc = np.stack(
        [
            cb_eff,
            2 * cb_eff,
            cb_eff * cb_eff,
            gn_scale * (2 - gn_scale),
            gn_scale,
            gn_weight,
            gn_bias,
        ],
        axis=1,
    ).astype(np.float32)  # [L, 7, C]

    xt0 = np.zeros((NC, D, NSH), np.float32)
    for r in range(NC):
        xt0[r] = x[r * NSH : (r + 1) * NSH].T
    xt0 = _bf(xt0)

    for c in range(NC):
        in_maps[c]["wts"] = wts
        in_maps[c]["attb"] = attb
        in_maps[c]["gnc"] = gnc
        in_maps[c]["xt0"] = xt0
        xtme = np.zeros((D, RW), np.float32)
        xtme[:, :NSH] = x[c * NSH : (c + 1) * NSH].T
        in_maps[c]["xtme0"] = _bf(xtme)

    cfg = dict(
        N=N, NSH=NSH, NBLK=NBLK, RW=RW, nchunk=[int(v) for v in nchunk],
        cum=[int(v) for v in cum], IWC=IWC,
    )
    return cfg, in_maps


def _ap3(ap, d1, d2):
    """Build [P, d1, d2] AP from a 2D AP by appending explicit dims."""
    return bass.AP(tensor=ap.tensor, offset=ap.offset, ap=[list(ap.ap[0]), d1, d2])


def build(cfg):
    N, NSH, NBLK, RW = cfg["N"], cfg["NSH"], cfg["NBLK"], cfg["RW"]
    nchunk, cum, IWC = cfg["nchunk"], cfg["cum"], cfg["IWC"]
    NT = N + (-N) % P
    nRT = (NSH + P - 1) // P
    NFULL = NSH // P  # full 128-row tiles per shard
    NREM = NSH - NFULL * P  # rows in the last partial tile

    nc = bacc.Bacc(
        "TRN2",
        target_bir_lowering=False,
        debug=False,
        num_devices=NC,
        num_swdge_queues=NQ,
        dynamic_dma_scratch_size=16384,
    )

    srcw = nc.dram_tensor("srcw", [P, IWC * 8], I16, kind="ExternalInput").ap()
    stm = nc.dram_tensor("stm", [P, IWC * 2 * P], BF16, kind="ExternalInput").ap()
    wts = nc.dram_tensor("wts", [L, 2, D + 1, HC], BF16, kind="ExternalInput").ap()
    attb = nc.dram_tensor("attb", [L, HC], BF16, kind="ExternalInput").ap()
    gnc = nc.dram_tensor("gnc", [L, 7, C], F32, kind="ExternalInput").ap()
    xt0 = nc.dram_tensor("xt0", [NC, D, NSH], BF16, kind="ExternalInput").ap()
    xtme0 = nc.dram_tensor("xtme0", [D, RW], BF16, kind="ExternalInput").ap()
    out = nc.dram_tensor("out", [NSH, C], F32, kind="ExternalOutput").ap()

    tab = nc.dram_tensor("tab", [NT, HC], BF16).ap()
    arin = [nc.dram_tensor(f"arin{l}", [P], F32).ap() for l in range(L)]
    arout = [
        nc.dram_tensor(f"arout{l}", [P], F32, addr_space="Shared").ap()
        for l in range(L)
    ]
    HSH = (NSH + 1) // 2
    agin2 = [
        [
            nc.dram_tensor(
                f"agin{l}h{h}", [D, HSH if h == 0 else NSH - HSH], BF16
            ).ap()
            for h in range(2)
        ]
        for l in range(L - 1)
    ]
    agout2 = [
        [
            nc.dram_tensor(
                f"agout{l}h{h}", [NC, D, HSH if h == 0 else NSH - HSH],
                BF16, addr_space="Shared",
            ).ap()
            for h in range(2)
        ]
        for l in range(L - 1)
    ]

    def _tab_out_ap(row0, pcnt, nb):
        """DRAM AP over tab rows row0 + b*128 + p (p outer, then b, then c)."""
        return bass.AP(
            tensor=tab.tensor,
            offset=tab.offset + row0 * HC,
            ap=[[HC, pcnt], [P * HC, nb], [1, HC]],
        )

    with tile.TileContext(nc) as tc:
        with (
            tc.tile_pool(name="res", bufs=1) as res,
            tc.tile_pool(name="stg", bufs=3) as stg,
            tc.tile_pool(name="big", bufs=3) as big,
            tc.tile_pool(name="gat", bufs=3) as gat,
            tc.tile_pool(name="idx", bufs=3) as idxp,
            tc.tile_pool(name="xtp", bufs=2) as xtp,
            tc.tile_pool(name="msk", bufs=3) as msk,
            tc.tile_pool(name="med", bufs=1) as med,
            tc.tile_pool(name="sm", bufs=2) as sm,
            tc.tile_pool(name="ps", bufs=2, space="PSUM") as ps,
            tc.tile_pool(name="psa", bufs=2, space="PSUM") as psa,
            tc.tile_pool(name="psb", bufs=1, space="PSUM") as psb,
            tc.tile_pool(name="psx", bufs=3, space="PSUM") as psx,
        ):
            # ---- resident loads / constants ----

            ones_row = res.tile([1, P], BF16)
            nc.vector.memset(ones_row[:], 1.0)
            ones_col = res.tile([P, 1], F32)
            nc.vector.memset(ones_col[:], 1.0)
            ident = res.tile([P, P], F32)
            make_identity(nc, ident[:])
            ident_bf = res.tile([P, P], BF16)
            nc.vector.tensor_copy(out=ident_bf[:], in_=ident[:])
            eps_col = res.tile([P, 1], F32)
            nc.vector.memset(eps_col[:], EPS)



            w_tiles = {}
            b_tiles = {}
            for l in range(L):
                for side in range(2):
                    # weights duplicated into both partition halves so lhsT
                    # slices based at partition 0 or 64 both find a matching
                    # rhs base
                    t = res.tile([P, HC], BF16, tag=f"w{l}{side}")
                    nc.sync.dma_start(out=t[:D, :], in_=wts[l, side, :D, :])
                    nc.sync.dma_start(out=t[D:, :], in_=wts[l, side, :D, :])
                    w_tiles[(l, side)] = t
                    if side == 1:
                        bt = res.tile([1, HC], BF16, tag=f"b{l}{side}")
                        nc.sync.dma_start(
                            out=bt[:], in_=wts[l, side, D : D + 1, :]
                        )
                        b_tiles[(l, side)] = bt

            att_bc = {}
            for l in range(L):
                t = res.tile([P, HC], BF16, tag=f"att{l}")
                nc.sync.dma_start(
                    out=t[:],
                    in_=bass.AP(
                        tensor=attb.tensor, offset=attb.offset + l * HC,
                        ap=[[0, P], [1, HC]],
                    ),
                )
                att_bc[l] = t

            gnc_bc = {}
            for l in range(L):
                t = res.tile([P, 7, C], F32, tag=f"gnc{l}")
                nc.sync.dma_start(
                    out=t[:],
                    in_=bass.AP(
                        tensor=gnc.tensor, offset=gnc.offset + l * 7 * C,
                        ap=[[0, P], [C, 7], [1, C]],
                    ),
                )
                gnc_bc[l] = t

            xr_res = res.tile([P, NBLK, HC], BF16)
            h_big = res.tile([P, NBLK, 2, C], F32)
            xtsh_sb = res.tile([D, RW], BF16)
            nc.vector.memset(xtsh_sb[:], 0.0)
            nc.sync.dma_start(out=xtsh_sb[:, :NSH], in_=xtme0[:, :NSH])

            for l in range(L):
                # ================= projections =================
                # xr first: it only needs own-shard features (xtsh_sb), so
                # on layer boundaries PE runs it while the AllGather that
                # feeds the xl loop below is still in flight.
                for j in range(nRT):
                    n0 = j * P
                    lhsT = xtsh_sb[:, n0 : n0 + P]
                    pt = ps.tile([P, HC], F32, tag="pj", space="PSUM")
                    nc.tensor.matmul(
                        out=pt[:], lhsT=lhsT, rhs=w_tiles[(l, 1)][:D, :],
                        start=True, stop=False,
                    )
                    nc.tensor.matmul(
                        out=pt[:], lhsT=ones_row[:],
                        rhs=b_tiles[(l, 1)][:],
                        start=False, stop=True,
                    )
                    if j % 2 == 0:
                        nc.scalar.activation(xr_res[:, j, :], pt[:], AF.Copy)
                    else:
                        nc.vector.tensor_copy(out=xr_res[:, j, :], in_=pt[:])

                WCH = 6
                for r in range(NC):
                    g0 = r * NSH
                    xt_r = xtp.tile([D, RW], BF16, tag="xtr")
                    if l == 0:
                        nc.sync.dma_start(out=xt_r[:, :NSH], in_=xt0[r, :, :])
                    else:
                        nc.sync.dma_start(
                            out=xt_r[:, :HSH],
                            in_=agout2[l - 1][0][r, :, :],
                        )
                        nc.sync.dma_start(
                            out=xt_r[:, HSH:NSH],
                            in_=agout2[l - 1][1][r, :, :],
                        )
                    for w0 in range(0, nRT, WCH):
                        wn = min(WCH, nRT - w0)
                        xl_st = stg.tile([P, WCH, HC], BF16, tag="pst")
                        for t in range(wn):
                            j = w0 + t
                            n0 = j * P
                            lhsT = xt_r[:, n0 : n0 + P]
                            pt = ps.tile([P, HC], F32, tag="pj", space="PSUM")
                            nc.tensor.matmul(
                                out=pt[:], lhsT=lhsT,
                                rhs=w_tiles[(l, 0)][:D, :],
                                start=True, stop=True,
                            )
                            if j % 2 == 0:
                                nc.scalar.activation(
                                    xl_st[:, t, :], pt[:], AF.Copy
                                )
                            else:
                                nc.vector.tensor_copy(
                                    out=xl_st[:, t, :], in_=pt[:]
                                )
                        wfull = min(wn, NFULL - w0)
                        if wfull > 0:
                            nc.sync.dma_start(
                                out=_tab_out_ap(g0 + w0 * P, P, wfull),
                                in_=xl_st[:, :wfull, :],
                            )
                        if w0 + wn > NFULL and NREM:
                            nc.sync.dma_start(
                                out=_tab_out_ap(g0 + NFULL * P, NREM, 1),
                                in_=xl_st[:NREM, NFULL - w0, :],
                            )

                # ================= edge blocks =================
                stats_ps = psb.tile([P, 1], F32, tag="stats", space="PSUM")
                gq = 0
                for b in range(NBLK):
                    nch = nchunk[b]
                    co = cum[b]

                    # gather xl[src] (chunks 0..nch) and xr[dst] (nch..2nch)
                    # dma_gather tops out at 1024 indices -- split into
                    # sub-calls (xl/xr interleaved per 8 chunks, see
                    # preprocess), round-robin across the 4 SWDGE queues
                    xl_g = gat.tile([P, nch, HC], BF16, tag="ug")
                    idx_sb = idxp.tile([P, nch * 8], I16, tag="ix")
                    nc.scalar.dma_start(
                        out=idx_sb[:],
                        in_=srcw[:, co * 8 : (co + nch) * 8],
                    )
                    for g in range(0, nch, 8):
                        gn = min(8, nch - g)
                        sub = gn * P
                        nc.gpsimd.dma_gather(
                            out_ap=xl_g[:, g : g + gn, :], in_ap=tab[:, :],
                            idxs_ap=idx_sb[:, g * 8 : (g + gn) * 8],
                            num_idxs=sub, num_idxs_reg=sub, elem_size=HC,
                            queue_num=gq % NQ,
                        )
                        gq += 1

                    # host-precomputed one-hot dst masks: s[i,k,e] | st[e,k,i]
                    mk = msk.tile([P, 2 * nch, P], BF16, tag="mk")
                    mo = co * 2 * P
                    nc.sync.dma_start(
                        out=mk[:, :nch, :],
                        in_=stm[:, mo : mo + nch * P],
                    )
                    nc.sync.dma_start(
                        out=mk[:, nch:, :],
                        in_=stm[:, mo + nch * P : mo + 2 * nch * P],
                    )

                    # u = xl[src] + xr[dst] accumulated in PSUM via identity
                    # matmuls, then leaky-relu straight from PSUM to SBUF
                    lr = big.tile([P, nch, HC], BF16, tag="g1")
                    for j in range(0, nch, 2):
                        jn = min(2, nch - j)
                        ups = psx.tile([P, 2, HC], F32, tag="xre", space="PSUM")
                        # one wide identity matmul folds xl for both chunks,
                        # then per-chunk mask matmuls add the xr expansion
                        nc.tensor.matmul(
                            out=ups[:, :jn, :].rearrange("p a b -> p (a b)"),
                            lhsT=ident_bf[:],
                            rhs=xl_g[:, j : j + jn, :].rearrange(
                                "p a b -> p (a b)"
                            ),
                            start=True, stop=False,
                        )
                        for t in range(jn):
                            nc.tensor.matmul(
                                out=ups[:, t, :], lhsT=mk[:, j + t, :],
                                rhs=xr_res[:, b, :],
                                start=False, stop=(t == jn - 1),
                            )
                        nc.scalar.activation(
                            lr[:, j : j + jn, :], ups[:, :jn, :],
                            AF.Prelu, alpha=NEG,
                        )

                    v = big.tile([P, nch, HC], BF16, tag="g2")
                    ab = att_bc[l][:]
                    nc.vector.tensor_mul(
                        out=v[:], in0=lr[:], in1=_ap3(ab, [0, nch], [1, HC])
                    )
                    logits = sm.tile([P, nch, H], F32, tag="lg")
                    # tree-reduce over C: 64 -> 32 -> 16 via 2x-mode TT adds,
                    # then one 1x tensor_reduce over the remaining 16
                    # r1/r2 scratch lives in lr, which is dead after the
                    # v = lr*att multiply above
                    r1 = lr[:, :, 0:128].rearrange("p n (h c) -> p n h c", h=H)
                    r2 = lr[:, :, 128:192].rearrange(
                        "p n (h c) -> p n h c", h=H
                    )
                    vv = v[:].rearrange("p n (h c) -> p n h c", h=H)
                    nc.vector.tensor_tensor(
                        out=r1, in0=vv[:, :, :, :32], in1=vv[:, :, :, 32:],
                        op=ALU.add,
                    )
                    nc.vector.tensor_tensor(
                        out=r2, in0=r1[:, :, :, :16], in1=r1[:, :, :, 16:],
                        op=ALU.add,
                    )
                    nc.vector.tensor_reduce(
                        out=logits[:],
                        in_=r2,
                        axis=mybir.AxisListType.X,
                        op=ALU.add,
                    )
                    # wcat: cols 0:H hold a=exp(logits), cols H: hold a*xl
                    wcat = big.tile([P, nch, H + HC], BF16, tag="g2")
                    nc.scalar.activation(wcat[:, :, :H], logits[:], AF.Exp)
                    # a2: each a value duplicated twice (contiguous) so the
                    # per-head multiply below reads it with an inner [1,2]
                    # stride pattern -> DVE 2x mode
                    a2 = sm.tile([P, nch, H, 2], BF16, tag="a2")
                    lg_ap = logits[:]
                    nc.scalar.activation(
                        a2[:],
                        bass.AP(
                            tensor=lg_ap.tensor, offset=lg_ap.offset,
                            ap=[list(lg_ap.ap[0]), [H, nch], [1, H], [0, 2]],
                        ),
                        AF.Exp,
                    )
                    wc_ap = wcat[:]
                    ug_ap = xl_g[:]
                    a2_ap = a2[:]
                    for h in range(H):
                        nc.vector.tensor_mul(
                            out=bass.AP(
                                tensor=wc_ap.tensor,
                                offset=wc_ap.offset + H + h * C,
                                ap=[list(wc_ap.ap[0]), [H + HC, nch],
                                    [2, C // 2], [1, 2]],
                            ),
                            in0=bass.AP(
                                tensor=ug_ap.tensor, offset=ug_ap.offset + h * C,
                                ap=[list(ug_ap.ap[0]), [HC, nch],
                                    [2, C // 2], [1, 2]],
                            ),
                            in1=bass.AP(
                                tensor=a2_ap.tensor, offset=a2_ap.offset + h * 2,
                                ap=[list(a2_ap.ap[0]), [2 * H, nch],
                                    [0, C // 2], [1, 2]],
                            ),
                        )

                    agg_ps = psa.tile([P, H + HC], F32, tag="agg", space="PSUM")
                    for j in range(nch):
                        nc.tensor.matmul(
                            out=agg_ps[:], lhsT=mk[:, nch + j, :],
                            rhs=wcat[:, j, :],
                            start=(j == 0), stop=(j == nch - 1),
                        )

                    # epilogue: h_blk = mean_h(agg/den) (conv_bias folded
                    # into the GraphNorm affine)
                    den4 = sm.tile([P, H], F32, tag="d4")
                    nc.scalar.activation(
                        den4[:], agg_ps[:, :H], AF.Copy, scale=float(H),
                        bias=1e-12,
                    )
                    rec4 = sm.tile([P, H], F32, tag="rc")
                    nc.vector.reciprocal(out=rec4[:], in_=den4[:])
                    sc = sm.tile([P, HC], F32, tag="sc")
                    nc.vector.tensor_mul(
                        out=sc[:].rearrange("p (h c) -> p h c", h=H),
                        in0=agg_ps[:, H:].rearrange("p (h c) -> p h c", h=H),
                        in1=rec4[:].to_broadcast([P, H, C]),
                    )
                    nc.vector.tensor_reduce(
                        out=h_big[:, b, 0, :],
                        in_=_ap3(sc[:], [1, C], [C, H]),
                        axis=mybir.AxisListType.X,
                        op=ALU.add,
                    )
                    nc.scalar.activation(
                        h_big[:, b, 1, :], h_big[:, b, 0, :], AF.Square
                    )
                # ================= GraphNorm =================
                # stats (column sums of [h | h^2]) in one PE chain, off the
                # per-block critical path
                for b in range(NBLK):
                    nc.tensor.matmul(
                        out=stats_ps[:],
                        lhsT=h_big[:, b, :, :].rearrange("p a b -> p (a b)"),
                        rhs=ones_col[:],
                        start=(b == 0), stop=(b == NBLK - 1),
                    )
                stats_sb = sm.tile([P, 1], F32, tag="stsb")
                nc.scalar.activation(stats_sb[:], stats_ps[:], AF.Copy)
                nc.scalar.dma_start(out=arin[l][:, None], in_=stats_sb[:])
                nc.gpsimd.collective_compute(
                    "AllReduce", ALU.add,
                    ins=[arin[l].opt()], outs=[arout[l].opt()],
                    replica_groups=[list(range(NC))],
                )
                srow = sm.tile([P, P], F32, tag="srow")
                nc.scalar.dma_start(
                    out=srow[:],
                    in_=bass.AP(
                        tensor=arout[l].tensor, offset=arout[l].offset,
                        ap=[[0, P], [1, P]],
                    ),
                )
                g = gnc_bc[l]
                invN = 1.0 / float(N)
                m12 = sm.tile([P, 2, C], F32, tag="m12")
                nc.scalar.activation(m12[:], srow[:, : 2 * C], AF.Copy, scale=invN)
                m1 = m12[:, 0, :]
                m2 = m12[:, 1, :]
                mu = sm.tile([P, C], F32, tag="mu")
                nc.vector.tensor_add(out=mu[:], in0=m1, in1=g[:, 0, :])
                t1 = sm.tile([P, C], F32, tag="t1")
                nc.vector.tensor_mul(out=t1[:], in0=mu[:], in1=mu[:])
                t2 = sm.tile([P, C], F32, tag="t2")
                nc.vector.tensor_mul(out=t2[:], in0=t1[:], in1=g[:, 3, :])
                u1 = sm.tile([P, C], F32, tag="u1")
                nc.vector.tensor_mul(out=u1[:], in0=m1, in1=g[:, 1, :])
                eh2 = sm.tile([P, C], F32, tag="eh2")
                nc.vector.tensor_add(out=eh2[:], in0=m2, in1=u1[:])
                nc.vector.tensor_add(out=eh2[:], in0=eh2[:], in1=g[:, 2, :])
                var = sm.tile([P, C], F32, tag="var")
                nc.vector.tensor_tensor(
                    out=var[:], in0=eh2[:], in1=t2[:], op=ALU.subtract
                )
                srt = sm.tile([P, C], F32, tag="srt")
                nc.scalar.activation(srt[:], var[:], AF.Sqrt, bias=eps_col[:])
                rst = sm.tile([P, C], F32, tag="rst")
                nc.vector.reciprocal(out=rst[:], in_=srt[:])
                A = sm.tile([P, C], F32, tag="A")
                nc.vector.tensor_mul(out=A[:], in0=rst[:], in1=g[:, 5, :])
                q = sm.tile([P, C], F32, tag="q")
                nc.vector.tensor_mul(out=q[:], in0=mu[:], in1=g[:, 4, :])
                nc.vector.tensor_tensor(
                    out=q[:], in0=g[:, 0, :], in1=q[:], op=ALU.subtract
                )
                Bt = sm.tile([P, C], F32, tag="B")
                nc.vector.tensor_mul(out=Bt[:], in0=A[:], in1=q[:])
                nc.vector.tensor_add(out=Bt[:], in0=Bt[:], in1=g[:, 6, :])

                # batched affine over all blocks, then per-block out/transpose
                xball = med.tile([P, NBLK, C], F32, tag="gn")
                nc.vector.tensor_mul(
                    out=xball[:], in0=h_big[:, :, 0, :],
                    in1=_ap3(A[:], [0, NBLK], [1, C]),
                )
                nc.vector.tensor_add(
                    out=xball[:], in0=xball[:],
                    in1=_ap3(Bt[:], [0, NBLK], [1, C]),
                )
                if l == L - 1:
                    for b in range(NBLK):
                        cnt = min(P, NSH - b * P)
                        nc.sync.dma_start(
                            out=out[b * P : b * P + cnt, :],
                            in_=xball[:cnt, b, :],
                        )
                else:
                    for b in range(0, NBLK, 2):
                        bn = min(2, NBLK - b)
                        tp = ps.tile([D, 2, P], F32, tag="pj", space="PSUM")
                        for t in range(bn):
                            nc.tensor.transpose(
                                out=tp[:, t, :], in_=xball[:, b + t, :],
                                identity=ident[:],
                            )
                        nc.scalar.activation(
                            xtsh_sb[:, b * P : (b + bn) * P], tp[:, :bn, :],
                            AF.Copy,
                        )
                if l < L - 1:
                    # two half-shard AllGathers: the first can start while the
                    # second half's transposes are still finishing
                    nc.scalar.dma_start(
                        out=agin2[l][0][:, :], in_=xtsh_sb[:, :HSH]
                    )
                    nc.gpsimd.collective_compute(
                        "AllGather", ALU.bypass,
                        ins=[agin2[l][0].opt()],
                        outs=[agout2[l][0].opt()],
                        replica_groups=[list(range(NC))],
                    )
                    nc.scalar.dma_start(
                        out=agin2[l][1][:, :], in_=xtsh_sb[:, HSH:NSH]
                    )
                    nc.gpsimd.collective_compute(
                        "AllGather", ALU.bypass,
                        ins=[agin2[l][1].opt()],
                        outs=[agout2[l][1].opt()],
                        replica_groups=[list(range(NC))],
                    )

    nc.compile()
    return nc


_CACHE = {}


def kernel(**inputs):
    cfg, in_maps = preprocess(inputs)
    key = (cfg["N"], tuple(cfg["nchunk"]))
    if key not in _CACHE:
        _CACHE[key] = build(cfg)
    nc = _CACHE[key]
    res = run_bass_kernel_spmd(nc, in_maps, core_ids=list(range(NC)))
    shards = [res.results[c]["out"] for c in range(NC)]
    return np.concatenate(shards, axis=0).astype(np.float32)


def _install_ntff_hook():
    import sys, types
    try:
        from antenv.axon_hooks import get_axon_ntff_profile_hook  # noqa
        return
    except ImportError:
        pass
    import trn_agent_boot.trn_boot as tb
    mod = types.ModuleType("antenv.axon_hooks")
    _hook = [None]
    mod.set_axon_ntff_profile_hook = lambda h: _hook.__setitem__(0, h)
    mod.get_axon_ntff_profile_hook = lambda: _hook[0]
    sys.modules["antenv.axon_hooks"] = mod
    import antenv
    antenv.axon_hooks = mod
    mod.set_axon_ntff_profile_hook(
        tb._ntff_profile_via_ctypes("/opt/axon/libaxon_pjrt.so")
    )


def run_traced(**inputs):
    """Re-run the cached kernel with NTFF tracing; returns exec_time_ns."""
    _install_ntff_hook()
    cfg, in_maps = preprocess(inputs)
    key = (cfg["N"], tuple(cfg["nchunk"]))
    if key not in _CACHE:
        _CACHE[key] = build(cfg)
    nc = _CACHE[key]
    res = run_bass_kernel_spmd(
        nc, in_maps, core_ids=list(range(NC)), trace=True
    )
    return res.exec_time_ns
